# revision 60
# baseline (speedup 1.0000x reference)
"""AWQ 4-bit quantized linear (group size 128) on 8 Trainium2 NeuronCores.

Column-parallel: each core owns OUT/8 = 1376 output columns. The host does
layout-only prep (slicing, int4->uint8 nibble widening with the AWQ column
permutation, dtype widening of the 4-bit zeros, transposes); all
arithmetic - zero-point subtract, scale multiply, matmul, bias - runs on
device.

v5: transpose-free dequant. The v3/v4 designs dequantized in [o, k] layout
and DMA-xbar-transposed to [k, o] for the PE's stationary operand. Trace
analysis showed the FIRST DMA_TRANSPOSE is serialized behind every
in-flight regular DMA copy (xbar-mode transition guard), so the whole
dequant->matmul pipeline idled until the full ~14MB front stream landed
(first real matmul at 45us of a 210us kernel).

v5 instead loads qweight pre-transposed from HBM ([k-within-group on
partitions, (group, out-col) on the free axis]) and dequantizes directly
in PE layout:
    w3[p,(g,o)] = (nib[p,(g,o)] - zB[p,(g,o)]) * sB[p,(g,o)]
where zB/sB are the per-(group,out-col) zero/scale rows replicated to all
128 partitions by plain stride-0 broadcast DMA reads from DRAM (no xbar
mode, no serialization). Two DVE tensor_tensor ops per dequant chunk.

Schedule (measured ~181us vs the ~152us pure-matmul roofline):
  - ALL loads ride the single sync HWDGE ring in exact need order. One
    FIFO ring sequences deliveries perfectly; concurrent rings round-robin
    per PACKET, so a big-packet ring starves a small-packet one (measured
    7:1), which is why a second load ring always lost.
  - ~28 warmup matmuls on a zeroed tile bridge the ~7us preamble plus the
    ~9us load prefix so the HAM clock gate reaches K=8/8 before real work
    and never re-throttles (every later PE pause < the ~3.4us window).
  - phase 1 interleaves otiles 0-3 block-major over the x chunks, riding
    the x frontier; dequant chunks are finest for otile 0 ([4,4,8,16]
    groups) and [8,8,16] for otiles 1-3 so the ramp starts early.
  - otiles 4-10 run sequentially, their qw + broadcasts queueing behind
    the phase-1 stream on the ring (prep two otiles ahead).
  - out stores ride the gpsimd SWDGE ring, except the last otile's which
    use the by-then-empty sync ring; the last otile runs 256-column
    quarter-chains (each in its own PSUM bank, evacuated immediately) so
    only one 256-col ACT + store is exposed after the final matmul.
Run-to-run note: the chip sometimes sits in a P0 power state with the PE
at 2.0 GHz instead of 2.4 (sustained-load downclock); the same binary
then measures ~213us instead of ~182us with identical scheduling.
"""

import os
import sys

import numpy as np

if "/opt/trn_rl_repo" not in sys.path:
    sys.path.insert(0, "/opt/trn_rl_repo")

M, IN, OUT = 1024, 4096, 11008
N_CORES = 8
OC = OUT // N_CORES  # 1376 output columns per core
GS = 128  # quantization group size (== matmul k-tile)
G = IN // GS  # 32 groups
PACK = 8  # int4 values per int32 word
# reference unpacks nibble k to logical column AWQ_REVERSE_ORDER.index(k);
# equivalently logical column j within a word uses shift 4*REV[j]:
REV = np.array([0, 4, 1, 5, 2, 6, 3, 7], dtype=np.uint32)

MM_N = 512  # moving-operand free size per matmul (one PSUM bank of fp32)
PHI = 4  # otiles interleaved in phase 1
XBLK = [4, 4, 8, 8, 8]  # x chunk sizes in groups == phase-1 block sizes
WARMUP_MMS = 28


def _chunk_plan(ot):
    """Dequant chunk sizes (in groups) per otile: fine-grained for the
    phase-1 otiles so the first matmuls start as early as possible and
    chunk boundaries align with the XBLK phase-1 blocks."""
    if ot == 0:
        return [4, 4, 8, 16]
    if 1 <= ot <= 3:
        return [8, 8, 16]
    return [16, 16]

_CACHE = {}


def _unpack_int4(q: np.ndarray) -> np.ndarray:
    """[rows, cols//8] int32 -> [rows, cols] uint8 in 0..15 (AWQ order)."""
    qu = q.view(np.uint32)
    nib = (qu[:, :, None] >> (REV * 4)[None, None, :]) & 0xF
    return nib.reshape(q.shape[0], -1).astype(np.uint8)


def _build(m, k, oc, n_cores):
    import concourse.bacc as bacc
    import concourse.tile as tile
    from concourse import mybir

    F16 = mybir.dt.float16
    F32 = mybir.dt.float32
    U8 = mybir.dt.uint8

    g = k // GS
    n_ot = (oc + 127) // 128
    n_mch = (m + MM_N - 1) // MM_N
    gk = g * 128  # free width of one otile: (group, out-col) flattened

    nc = bacc.Bacc("TRN2", target_bir_lowering=False, debug=False)
    # x pre-swizzled on host to [partition=k-within-group, group, m]
    x3 = nc.dram_tensor("x3", [128, g, m], F16, kind="ExternalInput").ap()
    # packed weights pre-transposed: [partition=k-within-group, otile, (g,o)]
    qwT = nc.dram_tensor("qwT", [128, n_ot, gk], U8, kind="ExternalInput").ap()
    # scales / zeros rows per otile, (g,o)-flattened; bias partition-major
    s2 = nc.dram_tensor("s2", [n_ot, gk], F16, kind="ExternalInput").ap()
    zq2 = nc.dram_tensor("zq2", [n_ot, gk], U8, kind="ExternalInput").ap()
    cb = nc.dram_tensor("cb", [128, n_ot], F16, kind="ExternalInput").ap()
    outT = nc.dram_tensor("outT", [oc, m], F16, kind="ExternalOutput").ap()

    with tile.TileContext(nc) as tc:
        with (
            tc.tile_pool(name="x", bufs=1) as xpool,
            tc.tile_pool(name="consts", bufs=1) as cpool,
            tc.tile_pool(name="warm", bufs=1) as warmpool,
            tc.tile_pool(name="qw", bufs=1) as qwpool,
            tc.tile_pool(name="b", bufs=5) as bpool,
            tc.tile_pool(name="w", bufs=10) as wpool,
            tc.tile_pool(name="ps", bufs=8, space="PSUM") as pspool,
            tc.tile_pool(name="o", bufs=4) as opool,
        ):
            # resident transposed activations: [128, g, m]
            xT_sb = xpool.tile([128, g, m], F16)

            def warmup():
                # lift the HAM clock gate to K=8/8 while the prologue DMAs
                # land: cold matmuls on a zeroed scratch tile keep the PE
                # busy >3.4us, so the real stream starts at full clock.
                wsrc = warmpool.tile([128, MM_N], F16, name="warm_src")
                wps = pspool.tile([128, MM_N], F32, name="warm_ps", tag="ps")
                nc.gpsimd.memset(wsrc[:], 0.0)
                for _ in range(WARMUP_MMS):
                    nc.tensor.matmul(
                        wps[:], wsrc[:, :128], wsrc[:], start=True, stop=True
                    )

            def load_consts():
                cb_sb = cpool.tile([128, n_ot], F16, tag="cb")
                nc.sync.dma_start(cb_sb[:], cb[:])
                return cb_sb

            def load_qw(ot):
                qw_t = qwpool.tile([128, 1, gk], U8, name=f"qw_{ot}", tag=f"qw{ot}")
                nc.sync.dma_start(qw_t[:], qwT[:, ot : ot + 1])
                qws[ot] = qw_t[:, 0]

            def load_qw_half(ot, h):
                # half-otile (16-group) weight loads keep the latency-
                # critical ring prefix small for the phase-1 otiles
                qw_t = qwpool.tile(
                    [128, 1, gk // 2], U8, name=f"qw_{ot}_{h}", tag=f"qw{ot}h{h}"
                )
                hw = gk // 2
                nc.sync.dma_start(qw_t[:], qwT[:, ot : ot + 1, h * hw : (h + 1) * hw])
                qwh[(ot, h)] = qw_t[:, 0]

            def qwsl(ot, g0, sz):
                if (ot, 0) in qwh:
                    h = (g0 * 128) // (gk // 2)
                    b = g0 * 128 - h * (gk // 2)
                    return qwh[(ot, h)][:, b : b + sz * 128]
                return qws[ot][:, g0 * 128 : (g0 + sz) * 128]

            def bcast_deq(ot, g0, sz, sub_eng=None):
                # replicate the zero / scale rows for groups [g0, g0+sz) of
                # otile ot to all 128 partitions with stride-0-source DMA
                # reads from DRAM (SBUF sources reject zero partition step),
                # then dequant w3 = (nib - z) * s with two elementwise passes
                # (subtract optionally on gpsimd to cut DVE-queue latency in
                # the phase-1 ramp; multiply always on DVE).
                w = sz * 128
                sl = slice(g0 * 128, g0 * 128 + w)
                zB_t = bpool.tile([128, 16 * 128], U8, tag="zB")
                sB_t = bpool.tile([128, 16 * 128], F16, tag="sB")
                nc.sync.dma_start(
                    zB_t[:, :w], zq2[ot : ot + 1, sl].partition_broadcast(128)
                )
                nc.sync.dma_start(
                    sB_t[:, :w], s2[ot : ot + 1, sl].partition_broadcast(128)
                )
                w3_t = wpool.tile(
                    [128, w], F16, tag=f"w{sz}", bufs={4: 2, 8: 7, 16: 10}[sz]
                )
                (sub_eng or nc.vector).tensor_tensor(
                    w3_t[:], qwsl(ot, g0, sz), zB_t[:, :w], mybir.AluOpType.subtract
                )
                nc.vector.tensor_tensor(
                    w3_t[:], w3_t[:], sB_t[:, :w], mybir.AluOpType.mult
                )
                w3s.setdefault(ot, []).append((g0, sz, w3_t))

            def prep(ot):
                load_qw(ot)
                for g0, sz in zip(
                    np.cumsum([0] + _chunk_plan(ot)[:-1]), _chunk_plan(ot)
                ):
                    bcast_deq(ot, int(g0), sz)

            def load_x(g0, sizes, eng):
                for sz in sizes:
                    eng.dma_start(xT_sb[:, g0 : g0 + sz], x3[:, g0 : g0 + sz])
                    g0 += sz
                return g0

            mslices = [
                slice(mc * MM_N, min(m, (mc + 1) * MM_N)) for mc in range(n_mch)
            ]

            def make_pss(ot):
                return [
                    pspool.tile([128, MM_N], F32, name=f"ps_{ot}_{mc}", tag="ps")
                    for mc in range(n_mch)
                ]

            def mm_one(ot, pss, gi, mc):
                msl = mslices[mc]
                for g0, sz, w3_t in w3s[ot]:
                    if g0 <= gi < g0 + sz:
                        j = gi - g0
                        break
                else:
                    raise AssertionError((ot, gi))
                nc.tensor.matmul(
                    pss[mc][:, : msl.stop - msl.start],
                    w3_t[:, j * 128 : (j + 1) * 128],
                    xT_sb[:, gi, msl],
                    start=(gi == 0),
                    stop=(gi == g - 1),
                )

            def evac(ot, pss, mcs=None, eng=None):
                o0 = ot * 128
                ob = min(128, oc - o0)
                b_t = cb_sb[:ob, ot : ot + 1]
                for mc in mcs if mcs is not None else range(n_mch):
                    msl = mslices[mc]
                    mn = msl.stop - msl.start
                    o_t = opool.tile([128, MM_N], F16, name="o_t", tag="o")
                    nc.scalar.activation(
                        o_t[:ob, :mn],
                        pss[mc][:ob, :mn],
                        mybir.ActivationFunctionType.Identity,
                        bias=b_t[:],
                        scale=1.0,
                    )
                    (eng or nc.gpsimd).dma_start(
                        outT[o0 : o0 + ob, msl], o_t[:ob, :mn]
                    )

            # ---- emission ----
            # ALL loads ride the single sync HWDGE ring in exact need
            # order: one FIFO ring sequences deliveries perfectly and there
            # is no cross-ring packet-size competition (rings round-robin
            # per packet, so a big-packet ring starves a small-packet one).
            # Only the out stores use a second (gpsimd SWDGE) ring.
            warmup()
            cb_sb = load_consts()
            w3s = {}
            qws = {}
            qwh = {}
            load_qw(0)
            gx = load_x(0, [2, 2], nc.sync)  # x block 0, split for finer gating
            bcast_deq(0, 0, 4)
            load_qw_half(1, 0)
            bcast_deq(1, 0, 8)
            load_qw_half(2, 0)
            bcast_deq(2, 0, 8)
            load_qw_half(3, 0)
            bcast_deq(3, 0, 8)
            bcast_deq(0, 4, 4)
            gx = load_x(gx, XBLK[1:2], nc.sync)  # x block 1
            bcast_deq(0, 8, 8)
            gx = load_x(gx, XBLK[2:3], nc.sync)  # x block 2
            bcast_deq(1, 8, 8)
            bcast_deq(2, 8, 8)
            bcast_deq(3, 8, 8)
            load_qw_half(1, 1)
            bcast_deq(0, 16, 16)
            load_qw_half(2, 1)
            load_qw_half(3, 1)
            gx = load_x(gx, XBLK[3:4], nc.sync)  # x block 3
            bcast_deq(1, 16, 16)
            bcast_deq(2, 16, 16)
            bcast_deq(3, 16, 16)
            load_x(gx, XBLK[4:], nc.sync)  # x block 4

            # phase 1: x-chunk-aligned bursts over otiles 0-3, riding the
            # x frontier
            pss_phi = {ot: make_pss(ot) for ot in range(PHI)}
            g0 = 0
            for sz in XBLK:
                for ot in range(PHI):
                    for gi in range(g0, g0 + sz):
                        for mc in range(n_mch):
                            mm_one(ot, pss_phi[ot], gi, mc)
                g0 += sz

            # prep the first steady otiles while phase 1 streams (their
            # qw + broadcast DMAs queue behind the x tail on the ring)
            prep(PHI)
            prep(PHI + 1)
            for ot in range(PHI):
                evac(ot, pss_phi[ot])

            # steady phase: otiles 4..n_otiles-1 sequential, mc-interleaved
            AHEAD = 2
            for ot in range(PHI, n_ot):
                nxt = ot + AHEAD
                if nxt < n_ot and nxt >= PHI + AHEAD:
                    prep(nxt)
                if ot == n_ot - 1:
                    # chain-major in 256-column quarter-chains, each in its
                    # own PSUM bank and evacuated immediately: the exposed
                    # tail after the very last matmul is a single 256-col
                    # ACT + store on the (empty by now) sync HWDGE ring
                    o0 = ot * 128
                    ob = min(128, oc - o0)
                    for q in range(2 * n_mch):
                        msl = slice(q * 256, q * 256 + 256)
                        ps_q = pspool.tile(
                            [128, 256], F32, name=f"ps_q{q}", tag="ps"
                        )
                        for gi in range(g):
                            for a, b, w3_t in w3s[ot]:
                                if a <= gi < a + b:
                                    j = gi - a
                                    break
                            nc.tensor.matmul(
                                ps_q[:],
                                w3_t[:, j * 128 : (j + 1) * 128],
                                xT_sb[:, gi, msl],
                                start=(gi == 0),
                                stop=(gi == g - 1),
                            )
                        o_t = opool.tile([128, MM_N], F16, name="o_t", tag="o")
                        nc.scalar.activation(
                            o_t[:ob, :256],
                            ps_q[:ob],
                            mybir.ActivationFunctionType.Identity,
                            bias=cb_sb[:ob, ot : ot + 1],
                            scale=1.0,
                        )
                        nc.sync.dma_start(outT[o0 : o0 + ob, msl], o_t[:ob, :256])
                else:
                    pss = make_pss(ot)
                    for gi in range(g):
                        for mc in range(n_mch):
                            mm_one(ot, pss, gi, mc)
                    evac(ot, pss)

    nc.compile()
    return nc


def _get_nc(m=M, k=IN, oc=OC, n_cores=N_CORES):
    key = (m, k, oc, n_cores)
    if key not in _CACHE:
        _CACHE[key] = _build(*key)
    return _CACHE[key]


def _make_in_maps(x, qweight, qzeros, scales, bias, n_cores=N_CORES):
    iw8 = _unpack_int4(qweight)  # [IN, OUT] uint8
    iz8 = _unpack_int4(qzeros)  # [G, OUT] uint8
    kk, mm = x.shape[1], x.shape[0]
    g = kk // GS
    # [p, group, m]: partition = k-within-group, contiguous per-partition
    x3 = np.ascontiguousarray(x.T.reshape(g, GS, mm).transpose(1, 0, 2))
    oc = qweight.shape[1] * PACK // n_cores
    n_ot = (oc + 127) // 128
    ocp = n_ot * 128

    in_maps = []
    for c in range(n_cores):
        sl = slice(c * oc, (c + 1) * oc)
        wp = np.pad(iw8[:, sl], [(0, 0), (0, ocp - oc)])  # [IN, ocp]
        # qwT[p, ot, (g, o)] = wp[g*GS + p, ot*128 + o]
        qwT = np.ascontiguousarray(
            wp.reshape(g, GS, n_ot, 128).transpose(1, 2, 0, 3).reshape(GS, n_ot, -1)
        )
        sp = np.pad(scales[:, sl].astype(np.float16), [(0, 0), (0, ocp - oc)])
        s2 = np.ascontiguousarray(
            sp.reshape(g, n_ot, 128).transpose(1, 0, 2).reshape(n_ot, -1)
        )
        zp = np.pad(iz8[:, sl], [(0, 0), (0, ocp - oc)])
        zq2 = np.ascontiguousarray(
            zp.reshape(g, n_ot, 128).transpose(1, 0, 2).reshape(n_ot, -1)
        )
        bp = np.pad(bias[sl].astype(np.float16), (0, ocp - oc))
        cb = np.ascontiguousarray(bp.reshape(n_ot, 128).T)
        in_maps.append({"x3": x3, "qwT": qwT, "s2": s2, "zq2": zq2, "cb": cb})
    return in_maps


LAST_EXEC_NS = None


def kernel(x, qweight, qzeros, scales, bias):
    global LAST_EXEC_NS
    from concourse.bass_utils import run_bass_kernel_spmd

    x = np.asarray(x)
    qweight = np.asarray(qweight)
    qzeros = np.asarray(qzeros)
    scales = np.asarray(scales)
    bias = np.asarray(bias)

    nc = _get_nc()
    in_maps = _make_in_maps(x, qweight, qzeros, scales, bias)

    kwargs = {}
    if os.environ.get("AWQ_PROFILE"):
        _enable_profiling()
        kwargs = dict(trace=True, tmpdir=os.environ.get("AWQ_TRACE_DIR") or None)
    res = run_bass_kernel_spmd(nc, in_maps, list(range(N_CORES)), **kwargs)
    LAST_EXEC_NS = res.exec_time_ns

    outT = np.concatenate([res.results[c]["outT"] for c in range(N_CORES)], axis=0)
    return np.ascontiguousarray(outT.T)


def _enable_profiling():
    """Register the NTFF profile hook missing from this image's antenv."""
    import types

    if "antenv.axon_hooks" not in sys.modules:
        import antenv

        mod = types.ModuleType("antenv.axon_hooks")
        mod._hook = None
        mod.set_axon_ntff_profile_hook = lambda h: setattr(mod, "_hook", h)
        mod.get_axon_ntff_profile_hook = lambda: mod._hook
        sys.modules["antenv.axon_hooks"] = mod
        antenv.axon_hooks = mod
        try:
            from trn_agent_boot.trn_boot import _ntff_profile_via_ctypes

            mod.set_axon_ntff_profile_hook(
                _ntff_profile_via_ctypes("/opt/axon/libaxon_pjrt.so")
            )
        except Exception:
            pass
    import concourse.bass_utils as _bu

    _bu.upload_artifacts = lambda tmpdir: "local://skipped"


# revision 61
# speedup vs baseline: 1.0006x; 1.0006x over previous
"""AWQ 4-bit quantized linear (group size 128) on 8 Trainium2 NeuronCores.

Column-parallel: each core owns OUT/8 = 1376 output columns. The host does
layout-only prep (slicing, int4->uint8 nibble widening with the AWQ column
permutation, dtype widening of the 4-bit zeros, transposes); all
arithmetic - zero-point subtract, scale multiply, matmul, bias - runs on
device.

v5: transpose-free dequant. The v3/v4 designs dequantized in [o, k] layout
and DMA-xbar-transposed to [k, o] for the PE's stationary operand. Trace
analysis showed the FIRST DMA_TRANSPOSE is serialized behind every
in-flight regular DMA copy (xbar-mode transition guard), so the whole
dequant->matmul pipeline idled until the full ~14MB front stream landed
(first real matmul at 45us of a 210us kernel).

v5 instead loads qweight pre-transposed from HBM ([k-within-group on
partitions, (group, out-col) on the free axis]) and dequantizes directly
in PE layout:
    w3[p,(g,o)] = (nib[p,(g,o)] - zB[p,(g,o)]) * sB[p,(g,o)]
where zB/sB are the per-(group,out-col) zero/scale rows replicated to all
128 partitions by plain stride-0 broadcast DMA reads from DRAM (no xbar
mode, no serialization). Two DVE tensor_tensor ops per dequant chunk.

Schedule (measured ~181us vs the ~152us pure-matmul roofline):
  - ALL loads ride the single sync HWDGE ring in exact need order. One
    FIFO ring sequences deliveries perfectly; concurrent rings round-robin
    per PACKET, so a big-packet ring starves a small-packet one (measured
    7:1), which is why a second load ring always lost.
  - ~28 warmup matmuls on a zeroed tile bridge the ~7us preamble plus the
    ~9us load prefix so the HAM clock gate reaches K=8/8 before real work
    and never re-throttles (every later PE pause < the ~3.4us window).
  - phase 1 interleaves otiles 0-3 block-major over the x chunks, riding
    the x frontier; dequant chunks are finest for otile 0 ([4,4,8,16]
    groups) and [8,8,16] for otiles 1-3 so the ramp starts early.
  - otiles 4-10 run sequentially, their qw + broadcasts queueing behind
    the phase-1 stream on the ring (prep two otiles ahead).
  - out stores ride the gpsimd SWDGE ring, except the last otile's which
    use the by-then-empty sync ring; the last otile runs 256-column
    quarter-chains (each in its own PSUM bank, evacuated immediately) so
    only one 256-col ACT + store is exposed after the final matmul.
Run-to-run note: the chip sometimes sits in a P0 power state with the PE
at 2.0 GHz instead of 2.4 (sustained-load downclock); the same binary
then measures ~213us instead of ~182us with identical scheduling.
"""

import os
import sys

import numpy as np

if "/opt/trn_rl_repo" not in sys.path:
    sys.path.insert(0, "/opt/trn_rl_repo")

M, IN, OUT = 1024, 4096, 11008
N_CORES = 8
OC = OUT // N_CORES  # 1376 output columns per core
GS = 128  # quantization group size (== matmul k-tile)
G = IN // GS  # 32 groups
PACK = 8  # int4 values per int32 word
# reference unpacks nibble k to logical column AWQ_REVERSE_ORDER.index(k);
# equivalently logical column j within a word uses shift 4*REV[j]:
REV = np.array([0, 4, 1, 5, 2, 6, 3, 7], dtype=np.uint32)

MM_N = 512  # moving-operand free size per matmul (one PSUM bank of fp32)
PHI = 4  # otiles interleaved in phase 1
XBLK = [4, 4, 8, 8, 8]  # x chunk sizes in groups == phase-1 block sizes
WARMUP_MMS = 28


def _chunk_plan(ot):
    """Dequant chunk sizes (in groups) per otile: fine-grained for the
    phase-1 otiles so the first matmuls start as early as possible and
    chunk boundaries align with the XBLK phase-1 blocks."""
    if ot == 0:
        return [4, 4, 8, 16]
    if 1 <= ot <= 3:
        return [8, 8, 16]
    return [16, 16]

_CACHE = {}


def _unpack_int4(q: np.ndarray) -> np.ndarray:
    """[rows, cols//8] int32 -> [rows, cols] uint8 in 0..15 (AWQ order)."""
    qu = q.view(np.uint32)
    nib = (qu[:, :, None] >> (REV * 4)[None, None, :]) & 0xF
    return nib.reshape(q.shape[0], -1).astype(np.uint8)


def _build(m, k, oc, n_cores):
    import concourse.bacc as bacc
    import concourse.tile as tile
    from concourse import mybir

    F16 = mybir.dt.float16
    F32 = mybir.dt.float32
    U8 = mybir.dt.uint8

    g = k // GS
    n_ot = (oc + 127) // 128
    n_mch = (m + MM_N - 1) // MM_N
    gk = g * 128  # free width of one otile: (group, out-col) flattened

    nc = bacc.Bacc("TRN2", target_bir_lowering=False, debug=False)
    # x pre-swizzled on host to [partition=k-within-group, group, m]
    x3 = nc.dram_tensor("x3", [128, g, m], F16, kind="ExternalInput").ap()
    # packed weights pre-transposed: [partition=k-within-group, otile, (g,o)]
    qwT = nc.dram_tensor("qwT", [128, n_ot, gk], U8, kind="ExternalInput").ap()
    # scales / zeros rows per otile, (g,o)-flattened; bias partition-major
    s2 = nc.dram_tensor("s2", [n_ot, gk], F16, kind="ExternalInput").ap()
    zq2 = nc.dram_tensor("zq2", [n_ot, gk], U8, kind="ExternalInput").ap()
    cb = nc.dram_tensor("cb", [128, n_ot], F16, kind="ExternalInput").ap()
    outT = nc.dram_tensor("outT", [oc, m], F16, kind="ExternalOutput").ap()

    with tile.TileContext(nc) as tc:
        with (
            tc.tile_pool(name="x", bufs=1) as xpool,
            tc.tile_pool(name="consts", bufs=1) as cpool,
            tc.tile_pool(name="warm", bufs=1) as warmpool,
            tc.tile_pool(name="qw", bufs=1) as qwpool,
            tc.tile_pool(name="b", bufs=5) as bpool,
            tc.tile_pool(name="w", bufs=10) as wpool,
            tc.tile_pool(name="ps", bufs=8, space="PSUM") as pspool,
            tc.tile_pool(name="o", bufs=4) as opool,
        ):
            # resident transposed activations: [128, g, m]
            xT_sb = xpool.tile([128, g, m], F16)

            def warmup():
                # lift the HAM clock gate to K=8/8 while the prologue DMAs
                # land: cold matmuls on a zeroed scratch tile keep the PE
                # busy >3.4us, so the real stream starts at full clock.
                wsrc = warmpool.tile([128, MM_N], F16, name="warm_src")
                wps = pspool.tile([128, MM_N], F32, name="warm_ps", tag="ps")
                nc.gpsimd.memset(wsrc[:], 0.0)
                for _ in range(WARMUP_MMS):
                    nc.tensor.matmul(
                        wps[:], wsrc[:, :128], wsrc[:], start=True, stop=True
                    )

            def load_consts():
                cb_sb = cpool.tile([128, n_ot], F16, tag="cb")
                nc.sync.dma_start(cb_sb[:], cb[:])
                return cb_sb

            def load_qw(ot):
                qw_t = qwpool.tile([128, 1, gk], U8, name=f"qw_{ot}", tag=f"qw{ot}")
                nc.sync.dma_start(qw_t[:], qwT[:, ot : ot + 1])
                qws[ot] = qw_t[:, 0]

            def load_qw_half(ot, h):
                # half-otile (16-group) weight loads keep the latency-
                # critical ring prefix small for the phase-1 otiles
                qw_t = qwpool.tile(
                    [128, 1, gk // 2], U8, name=f"qw_{ot}_{h}", tag=f"qw{ot}h{h}"
                )
                hw = gk // 2
                nc.sync.dma_start(qw_t[:], qwT[:, ot : ot + 1, h * hw : (h + 1) * hw])
                qwh[(ot, h)] = qw_t[:, 0]

            def qwsl(ot, g0, sz):
                if (ot, 0) in qwh:
                    h = (g0 * 128) // (gk // 2)
                    b = g0 * 128 - h * (gk // 2)
                    return qwh[(ot, h)][:, b : b + sz * 128]
                return qws[ot][:, g0 * 128 : (g0 + sz) * 128]

            def bcast_deq(ot, g0, sz, sub_eng=None):
                # replicate the zero / scale rows for groups [g0, g0+sz) of
                # otile ot to all 128 partitions with stride-0-source DMA
                # reads from DRAM (SBUF sources reject zero partition step),
                # then dequant w3 = (nib - z) * s with two elementwise passes
                # (subtract optionally on gpsimd to cut DVE-queue latency in
                # the phase-1 ramp; multiply always on DVE).
                w = sz * 128
                sl = slice(g0 * 128, g0 * 128 + w)
                zB_t = bpool.tile([128, 16 * 128], U8, tag="zB")
                sB_t = bpool.tile([128, 16 * 128], F16, tag="sB")
                nc.sync.dma_start(
                    zB_t[:, :w], zq2[ot : ot + 1, sl].partition_broadcast(128)
                )
                nc.sync.dma_start(
                    sB_t[:, :w], s2[ot : ot + 1, sl].partition_broadcast(128)
                )
                w3_t = wpool.tile(
                    [128, w], F16, tag=f"w{sz}", bufs={4: 2, 8: 7, 16: 10}[sz]
                )
                (sub_eng or nc.vector).tensor_tensor(
                    w3_t[:], qwsl(ot, g0, sz), zB_t[:, :w], mybir.AluOpType.subtract
                )
                nc.vector.tensor_tensor(
                    w3_t[:], w3_t[:], sB_t[:, :w], mybir.AluOpType.mult
                )
                w3s.setdefault(ot, []).append((g0, sz, w3_t))

            def prep(ot):
                load_qw(ot)
                for g0, sz in zip(
                    np.cumsum([0] + _chunk_plan(ot)[:-1]), _chunk_plan(ot)
                ):
                    bcast_deq(ot, int(g0), sz)

            def load_x(g0, sizes, eng):
                for sz in sizes:
                    eng.dma_start(xT_sb[:, g0 : g0 + sz], x3[:, g0 : g0 + sz])
                    g0 += sz
                return g0

            mslices = [
                slice(mc * MM_N, min(m, (mc + 1) * MM_N)) for mc in range(n_mch)
            ]

            def make_pss(ot):
                return [
                    pspool.tile([128, MM_N], F32, name=f"ps_{ot}_{mc}", tag="ps")
                    for mc in range(n_mch)
                ]

            def mm_one(ot, pss, gi, mc):
                msl = mslices[mc]
                for g0, sz, w3_t in w3s[ot]:
                    if g0 <= gi < g0 + sz:
                        j = gi - g0
                        break
                else:
                    raise AssertionError((ot, gi))
                nc.tensor.matmul(
                    pss[mc][:, : msl.stop - msl.start],
                    w3_t[:, j * 128 : (j + 1) * 128],
                    xT_sb[:, gi, msl],
                    start=(gi == 0),
                    stop=(gi == g - 1),
                )

            def evac(ot, pss, mcs=None, eng=None):
                o0 = ot * 128
                ob = min(128, oc - o0)
                b_t = cb_sb[:ob, ot : ot + 1]
                for mc in mcs if mcs is not None else range(n_mch):
                    msl = mslices[mc]
                    mn = msl.stop - msl.start
                    o_t = opool.tile([128, MM_N], F16, name="o_t", tag="o")
                    nc.scalar.activation(
                        o_t[:ob, :mn],
                        pss[mc][:ob, :mn],
                        mybir.ActivationFunctionType.Identity,
                        bias=b_t[:],
                        scale=1.0,
                    )
                    (eng or nc.gpsimd).dma_start(
                        outT[o0 : o0 + ob, msl], o_t[:ob, :mn]
                    )

            # ---- emission ----
            # ALL loads ride the single sync HWDGE ring in exact need
            # order: one FIFO ring sequences deliveries perfectly and there
            # is no cross-ring packet-size competition (rings round-robin
            # per packet, so a big-packet ring starves a small-packet one).
            # Only the out stores use a second (gpsimd SWDGE) ring.
            warmup()
            cb_sb = load_consts()
            w3s = {}
            qws = {}
            qwh = {}
            load_qw(0)
            gx = load_x(0, XBLK[:1], nc.sync)  # x block 0
            bcast_deq(0, 0, 4)
            load_qw_half(1, 0)
            bcast_deq(1, 0, 8)
            load_qw_half(2, 0)
            bcast_deq(2, 0, 8)
            load_qw_half(3, 0)
            bcast_deq(3, 0, 8)
            bcast_deq(0, 4, 4)
            gx = load_x(gx, XBLK[1:2], nc.sync)  # x block 1
            bcast_deq(0, 8, 8)
            gx = load_x(gx, XBLK[2:3], nc.sync)  # x block 2
            bcast_deq(1, 8, 8)
            bcast_deq(2, 8, 8)
            bcast_deq(3, 8, 8)
            load_qw_half(1, 1)
            bcast_deq(0, 16, 16)
            load_qw_half(2, 1)
            load_qw_half(3, 1)
            gx = load_x(gx, XBLK[3:4], nc.sync)  # x block 3
            bcast_deq(1, 16, 16)
            bcast_deq(2, 16, 16)
            bcast_deq(3, 16, 16)
            load_x(gx, XBLK[4:], nc.sync)  # x block 4

            # phase 1: x-chunk-aligned bursts over otiles 0-3, riding the
            # x frontier
            pss_phi = {ot: make_pss(ot) for ot in range(PHI)}
            g0 = 0
            for sz in XBLK:
                for ot in range(PHI):
                    for gi in range(g0, g0 + sz):
                        for mc in range(n_mch):
                            mm_one(ot, pss_phi[ot], gi, mc)
                    if g0 + sz == g:
                        # evacuate each otile right after its last matmul
                        # so its PSUM banks free for the steady otiles
                        evac(ot, pss_phi[ot])
                g0 += sz

            # prep the first steady otiles while phase 1 streams (their
            # qw + broadcast DMAs queue behind the x tail on the ring)
            prep(PHI)
            prep(PHI + 1)

            # steady phase: otiles 4..n_otiles-1 sequential, mc-interleaved
            AHEAD = 2
            for ot in range(PHI, n_ot):
                nxt = ot + AHEAD
                if nxt < n_ot and nxt >= PHI + AHEAD:
                    prep(nxt)
                if ot == n_ot - 1:
                    # chain-major in 256-column quarter-chains, each in its
                    # own PSUM bank and evacuated immediately: the exposed
                    # tail after the very last matmul is a single 256-col
                    # ACT + store on the (empty by now) sync HWDGE ring
                    o0 = ot * 128
                    ob = min(128, oc - o0)
                    for q in range(2 * n_mch):
                        msl = slice(q * 256, q * 256 + 256)
                        ps_q = pspool.tile(
                            [128, 256], F32, name=f"ps_q{q}", tag="ps"
                        )
                        for gi in range(g):
                            for a, b, w3_t in w3s[ot]:
                                if a <= gi < a + b:
                                    j = gi - a
                                    break
                            nc.tensor.matmul(
                                ps_q[:],
                                w3_t[:, j * 128 : (j + 1) * 128],
                                xT_sb[:, gi, msl],
                                start=(gi == 0),
                                stop=(gi == g - 1),
                            )
                        o_t = opool.tile([128, MM_N], F16, name="o_t", tag="o")
                        nc.scalar.activation(
                            o_t[:ob, :256],
                            ps_q[:ob],
                            mybir.ActivationFunctionType.Identity,
                            bias=cb_sb[:ob, ot : ot + 1],
                            scale=1.0,
                        )
                        nc.sync.dma_start(outT[o0 : o0 + ob, msl], o_t[:ob, :256])
                else:
                    pss = make_pss(ot)
                    for gi in range(g):
                        for mc in range(n_mch):
                            mm_one(ot, pss, gi, mc)
                    evac(ot, pss)

    nc.compile()
    return nc


def _get_nc(m=M, k=IN, oc=OC, n_cores=N_CORES):
    key = (m, k, oc, n_cores)
    if key not in _CACHE:
        _CACHE[key] = _build(*key)
    return _CACHE[key]


def _make_in_maps(x, qweight, qzeros, scales, bias, n_cores=N_CORES):
    iw8 = _unpack_int4(qweight)  # [IN, OUT] uint8
    iz8 = _unpack_int4(qzeros)  # [G, OUT] uint8
    kk, mm = x.shape[1], x.shape[0]
    g = kk // GS
    # [p, group, m]: partition = k-within-group, contiguous per-partition
    x3 = np.ascontiguousarray(x.T.reshape(g, GS, mm).transpose(1, 0, 2))
    oc = qweight.shape[1] * PACK // n_cores
    n_ot = (oc + 127) // 128
    ocp = n_ot * 128

    in_maps = []
    for c in range(n_cores):
        sl = slice(c * oc, (c + 1) * oc)
        wp = np.pad(iw8[:, sl], [(0, 0), (0, ocp - oc)])  # [IN, ocp]
        # qwT[p, ot, (g, o)] = wp[g*GS + p, ot*128 + o]
        qwT = np.ascontiguousarray(
            wp.reshape(g, GS, n_ot, 128).transpose(1, 2, 0, 3).reshape(GS, n_ot, -1)
        )
        sp = np.pad(scales[:, sl].astype(np.float16), [(0, 0), (0, ocp - oc)])
        s2 = np.ascontiguousarray(
            sp.reshape(g, n_ot, 128).transpose(1, 0, 2).reshape(n_ot, -1)
        )
        zp = np.pad(iz8[:, sl], [(0, 0), (0, ocp - oc)])
        zq2 = np.ascontiguousarray(
            zp.reshape(g, n_ot, 128).transpose(1, 0, 2).reshape(n_ot, -1)
        )
        bp = np.pad(bias[sl].astype(np.float16), (0, ocp - oc))
        cb = np.ascontiguousarray(bp.reshape(n_ot, 128).T)
        in_maps.append({"x3": x3, "qwT": qwT, "s2": s2, "zq2": zq2, "cb": cb})
    return in_maps


LAST_EXEC_NS = None


def kernel(x, qweight, qzeros, scales, bias):
    global LAST_EXEC_NS
    from concourse.bass_utils import run_bass_kernel_spmd

    x = np.asarray(x)
    qweight = np.asarray(qweight)
    qzeros = np.asarray(qzeros)
    scales = np.asarray(scales)
    bias = np.asarray(bias)

    nc = _get_nc()
    in_maps = _make_in_maps(x, qweight, qzeros, scales, bias)

    kwargs = {}
    if os.environ.get("AWQ_PROFILE"):
        _enable_profiling()
        kwargs = dict(trace=True, tmpdir=os.environ.get("AWQ_TRACE_DIR") or None)
    res = run_bass_kernel_spmd(nc, in_maps, list(range(N_CORES)), **kwargs)
    LAST_EXEC_NS = res.exec_time_ns

    outT = np.concatenate([res.results[c]["outT"] for c in range(N_CORES)], axis=0)
    return np.ascontiguousarray(outT.T)


def _enable_profiling():
    """Register the NTFF profile hook missing from this image's antenv."""
    import types

    if "antenv.axon_hooks" not in sys.modules:
        import antenv

        mod = types.ModuleType("antenv.axon_hooks")
        mod._hook = None
        mod.set_axon_ntff_profile_hook = lambda h: setattr(mod, "_hook", h)
        mod.get_axon_ntff_profile_hook = lambda: mod._hook
        sys.modules["antenv.axon_hooks"] = mod
        antenv.axon_hooks = mod
        try:
            from trn_agent_boot.trn_boot import _ntff_profile_via_ctypes

            mod.set_axon_ntff_profile_hook(
                _ntff_profile_via_ctypes("/opt/axon/libaxon_pjrt.so")
            )
        except Exception:
            pass
    import concourse.bass_utils as _bu

    _bu.upload_artifacts = lambda tmpdir: "local://skipped"


# revision 62
# speedup vs baseline: 1.0102x; 1.0096x over previous
"""AWQ 4-bit quantized linear (group size 128) on 8 Trainium2 NeuronCores.

Column-parallel: each core owns OUT/8 = 1376 output columns. The host does
layout-only prep (slicing, int4->uint8 nibble widening with the AWQ column
permutation, dtype widening of the 4-bit zeros, transposes); all
arithmetic - zero-point subtract, scale multiply, matmul, bias - runs on
device.

v5: transpose-free dequant. The v3/v4 designs dequantized in [o, k] layout
and DMA-xbar-transposed to [k, o] for the PE's stationary operand. Trace
analysis showed the FIRST DMA_TRANSPOSE is serialized behind every
in-flight regular DMA copy (xbar-mode transition guard), so the whole
dequant->matmul pipeline idled until the full ~14MB front stream landed
(first real matmul at 45us of a 210us kernel).

v5 instead loads qweight pre-transposed from HBM ([k-within-group on
partitions, (group, out-col) on the free axis]) and dequantizes directly
in PE layout:
    w3[p,(g,o)] = (nib[p,(g,o)] - zB[p,(g,o)]) * sB[p,(g,o)]
where zB/sB are the per-(group,out-col) zero/scale rows replicated to all
128 partitions by plain stride-0 broadcast DMA reads from DRAM (no xbar
mode, no serialization). Two DVE tensor_tensor ops per dequant chunk.

Schedule (measured ~181us vs the ~152us pure-matmul roofline):
  - ALL loads ride the single sync HWDGE ring in exact need order. One
    FIFO ring sequences deliveries perfectly; concurrent rings round-robin
    per PACKET, so a big-packet ring starves a small-packet one (measured
    7:1), which is why a second load ring always lost.
  - ~28 warmup matmuls on a zeroed tile bridge the ~7us preamble plus the
    ~9us load prefix so the HAM clock gate reaches K=8/8 before real work
    and never re-throttles (every later PE pause < the ~3.4us window).
  - phase 1 interleaves otiles 0-3 block-major over the x chunks, riding
    the x frontier; dequant chunks are finest for otile 0 ([4,4,8,16]
    groups) and [8,8,16] for otiles 1-3 so the ramp starts early.
  - otiles 4-10 run sequentially, their qw + broadcasts queueing behind
    the phase-1 stream on the ring (prep two otiles ahead).
  - out stores ride the gpsimd SWDGE ring, except the last otile's which
    use the by-then-empty sync ring; the last otile runs 256-column
    quarter-chains (each in its own PSUM bank, evacuated immediately) so
    only one 256-col ACT + store is exposed after the final matmul.
Run-to-run note: the chip sometimes sits in a P0 power state with the PE
at 2.0 GHz instead of 2.4 (sustained-load downclock); the same binary
then measures ~213us instead of ~182us with identical scheduling.
"""

import os
import sys

import numpy as np

if "/opt/trn_rl_repo" not in sys.path:
    sys.path.insert(0, "/opt/trn_rl_repo")

M, IN, OUT = 1024, 4096, 11008
N_CORES = 8
OC = OUT // N_CORES  # 1376 output columns per core
GS = 128  # quantization group size (== matmul k-tile)
G = IN // GS  # 32 groups
PACK = 8  # int4 values per int32 word
# reference unpacks nibble k to logical column AWQ_REVERSE_ORDER.index(k);
# equivalently logical column j within a word uses shift 4*REV[j]:
REV = np.array([0, 4, 1, 5, 2, 6, 3, 7], dtype=np.uint32)

MM_N = 512  # moving-operand free size per matmul (one PSUM bank of fp32)
PHI = 4  # otiles interleaved in phase 1
XBLK = [4, 4, 8, 8, 8]  # x chunk sizes in groups == phase-1 block sizes
WARMUP_MMS = 24


def _chunk_plan(ot):
    """Dequant chunk sizes (in groups) per otile: fine-grained for the
    phase-1 otiles so the first matmuls start as early as possible and
    chunk boundaries align with the XBLK phase-1 blocks."""
    if ot == 0:
        return [4, 4, 8, 16]
    if 1 <= ot <= 3:
        return [8, 8, 16]
    return [16, 16]

_CACHE = {}


def _unpack_int4(q: np.ndarray) -> np.ndarray:
    """[rows, cols//8] int32 -> [rows, cols] uint8 in 0..15 (AWQ order)."""
    qu = q.view(np.uint32)
    nib = (qu[:, :, None] >> (REV * 4)[None, None, :]) & 0xF
    return nib.reshape(q.shape[0], -1).astype(np.uint8)


def _build(m, k, oc, n_cores):
    import concourse.bacc as bacc
    import concourse.tile as tile
    from concourse import mybir

    F16 = mybir.dt.float16
    F32 = mybir.dt.float32
    U8 = mybir.dt.uint8

    g = k // GS
    n_ot = (oc + 127) // 128
    n_mch = (m + MM_N - 1) // MM_N
    gk = g * 128  # free width of one otile: (group, out-col) flattened

    nc = bacc.Bacc("TRN2", target_bir_lowering=False, debug=False)
    # x pre-swizzled on host to [partition=k-within-group, group, m]
    x3 = nc.dram_tensor("x3", [128, g, m], F16, kind="ExternalInput").ap()
    # packed weights pre-transposed: [partition=k-within-group, otile, (g,o)]
    qwT = nc.dram_tensor("qwT", [128, n_ot, gk], U8, kind="ExternalInput").ap()
    # scales / zeros rows per otile, (g,o)-flattened; bias partition-major
    s2 = nc.dram_tensor("s2", [n_ot, gk], F16, kind="ExternalInput").ap()
    zq2 = nc.dram_tensor("zq2", [n_ot, gk], U8, kind="ExternalInput").ap()
    cb = nc.dram_tensor("cb", [128, n_ot], F16, kind="ExternalInput").ap()
    outT = nc.dram_tensor("outT", [oc, m], F16, kind="ExternalOutput").ap()

    with tile.TileContext(nc) as tc:
        with (
            tc.tile_pool(name="x", bufs=1) as xpool,
            tc.tile_pool(name="consts", bufs=1) as cpool,
            tc.tile_pool(name="warm", bufs=1) as warmpool,
            tc.tile_pool(name="qw", bufs=1) as qwpool,
            tc.tile_pool(name="b", bufs=5) as bpool,
            tc.tile_pool(name="w", bufs=10) as wpool,
            tc.tile_pool(name="ps", bufs=8, space="PSUM") as pspool,
            tc.tile_pool(name="o", bufs=6) as opool,
        ):
            # resident transposed activations: [128, g, m]
            xT_sb = xpool.tile([128, g, m], F16)

            def warmup():
                # lift the HAM clock gate to K=8/8 while the prologue DMAs
                # land: cold matmuls on a zeroed scratch tile keep the PE
                # busy >3.4us, so the real stream starts at full clock.
                wsrc = warmpool.tile([128, MM_N], F16, name="warm_src")
                wps = pspool.tile([128, MM_N], F32, name="warm_ps", tag="ps")
                nc.gpsimd.memset(wsrc[:], 0.0)
                for _ in range(WARMUP_MMS):
                    nc.tensor.matmul(
                        wps[:], wsrc[:, :128], wsrc[:], start=True, stop=True
                    )

            def load_consts():
                cb_sb = cpool.tile([128, n_ot], F16, tag="cb")
                nc.sync.dma_start(cb_sb[:], cb[:])
                return cb_sb

            def load_qw(ot):
                qw_t = qwpool.tile([128, 1, gk], U8, name=f"qw_{ot}", tag=f"qw{ot}")
                nc.sync.dma_start(qw_t[:], qwT[:, ot : ot + 1])
                qws[ot] = qw_t[:, 0]

            def load_qw_half(ot, h):
                # half-otile (16-group) weight loads keep the latency-
                # critical ring prefix small for the phase-1 otiles
                qw_t = qwpool.tile(
                    [128, 1, gk // 2], U8, name=f"qw_{ot}_{h}", tag=f"qw{ot}h{h}"
                )
                hw = gk // 2
                nc.sync.dma_start(qw_t[:], qwT[:, ot : ot + 1, h * hw : (h + 1) * hw])
                qwh[(ot, h)] = qw_t[:, 0]

            def qwsl(ot, g0, sz):
                if (ot, 0) in qwh:
                    h = (g0 * 128) // (gk // 2)
                    b = g0 * 128 - h * (gk // 2)
                    return qwh[(ot, h)][:, b : b + sz * 128]
                return qws[ot][:, g0 * 128 : (g0 + sz) * 128]

            def bcast_deq(ot, g0, sz, sub_eng=None):
                # replicate the zero / scale rows for groups [g0, g0+sz) of
                # otile ot to all 128 partitions with stride-0-source DMA
                # reads from DRAM (SBUF sources reject zero partition step),
                # then dequant w3 = (nib - z) * s with two elementwise passes
                # (subtract optionally on gpsimd to cut DVE-queue latency in
                # the phase-1 ramp; multiply always on DVE).
                w = sz * 128
                sl = slice(g0 * 128, g0 * 128 + w)
                zB_t = bpool.tile([128, 16 * 128], U8, tag="zB")
                sB_t = bpool.tile([128, 16 * 128], F16, tag="sB")
                nc.sync.dma_start(
                    zB_t[:, :w], zq2[ot : ot + 1, sl].partition_broadcast(128)
                )
                nc.sync.dma_start(
                    sB_t[:, :w], s2[ot : ot + 1, sl].partition_broadcast(128)
                )
                w3_t = wpool.tile(
                    [128, w], F16, tag=f"w{sz}", bufs={4: 2, 8: 7, 16: 10}[sz]
                )
                (sub_eng or nc.vector).tensor_tensor(
                    w3_t[:], qwsl(ot, g0, sz), zB_t[:, :w], mybir.AluOpType.subtract
                )
                nc.vector.tensor_tensor(
                    w3_t[:], w3_t[:], sB_t[:, :w], mybir.AluOpType.mult
                )
                w3s.setdefault(ot, []).append((g0, sz, w3_t))

            def prep(ot):
                load_qw(ot)
                for g0, sz in zip(
                    np.cumsum([0] + _chunk_plan(ot)[:-1]), _chunk_plan(ot)
                ):
                    bcast_deq(ot, int(g0), sz)

            def load_x(g0, sizes, eng):
                for sz in sizes:
                    eng.dma_start(xT_sb[:, g0 : g0 + sz], x3[:, g0 : g0 + sz])
                    g0 += sz
                return g0

            mslices = [
                slice(mc * MM_N, min(m, (mc + 1) * MM_N)) for mc in range(n_mch)
            ]

            def make_pss(ot):
                return [
                    pspool.tile([128, MM_N], F32, name=f"ps_{ot}_{mc}", tag="ps")
                    for mc in range(n_mch)
                ]

            def mm_one(ot, pss, gi, mc):
                msl = mslices[mc]
                for g0, sz, w3_t in w3s[ot]:
                    if g0 <= gi < g0 + sz:
                        j = gi - g0
                        break
                else:
                    raise AssertionError((ot, gi))
                nc.tensor.matmul(
                    pss[mc][:, : msl.stop - msl.start],
                    w3_t[:, j * 128 : (j + 1) * 128],
                    xT_sb[:, gi, msl],
                    start=(gi == 0),
                    stop=(gi == g - 1),
                )

            def evac(ot, pss, mcs=None, eng=None):
                o0 = ot * 128
                ob = min(128, oc - o0)
                b_t = cb_sb[:ob, ot : ot + 1]
                for mc in mcs if mcs is not None else range(n_mch):
                    msl = mslices[mc]
                    mn = msl.stop - msl.start
                    o_t = opool.tile([128, MM_N], F16, name="o_t", tag="o")
                    nc.scalar.activation(
                        o_t[:ob, :mn],
                        pss[mc][:ob, :mn],
                        mybir.ActivationFunctionType.Identity,
                        bias=b_t[:],
                        scale=1.0,
                    )
                    (eng or nc.gpsimd).dma_start(
                        outT[o0 : o0 + ob, msl], o_t[:ob, :mn]
                    )

            # ---- emission ----
            # ALL loads ride the single sync HWDGE ring in exact need
            # order: one FIFO ring sequences deliveries perfectly and there
            # is no cross-ring packet-size competition (rings round-robin
            # per packet, so a big-packet ring starves a small-packet one).
            # Only the out stores use a second (gpsimd SWDGE) ring.
            warmup()
            cb_sb = load_consts()
            w3s = {}
            qws = {}
            qwh = {}
            load_qw(0)
            bcast_deq(0, 0, 4)
            gx = load_x(0, XBLK[:1], nc.sync)  # x block 0
            load_qw_half(1, 0)
            bcast_deq(1, 0, 8)
            load_qw_half(2, 0)
            bcast_deq(2, 0, 8)
            load_qw_half(3, 0)
            bcast_deq(3, 0, 8)
            bcast_deq(0, 4, 4)
            gx = load_x(gx, XBLK[1:2], nc.sync)  # x block 1
            bcast_deq(0, 8, 8)
            gx = load_x(gx, XBLK[2:3], nc.sync)  # x block 2
            bcast_deq(1, 8, 8)
            bcast_deq(2, 8, 8)
            bcast_deq(3, 8, 8)
            load_qw_half(1, 1)
            bcast_deq(0, 16, 16)
            load_qw_half(2, 1)
            load_qw_half(3, 1)
            gx = load_x(gx, XBLK[3:4], nc.sync)  # x block 3
            bcast_deq(1, 16, 16)
            bcast_deq(2, 16, 16)
            bcast_deq(3, 16, 16)
            load_x(gx, XBLK[4:], nc.sync)  # x block 4

            # phase 1: x-chunk-aligned bursts over otiles 0-3, riding the
            # x frontier
            pss_phi = {ot: make_pss(ot) for ot in range(PHI)}
            g0 = 0
            for sz in XBLK:
                for ot in range(PHI):
                    for gi in range(g0, g0 + sz):
                        for mc in range(n_mch):
                            mm_one(ot, pss_phi[ot], gi, mc)
                    if g0 + sz == g:
                        # evacuate each otile right after its last matmul
                        # so its PSUM banks free for the steady otiles
                        evac(ot, pss_phi[ot])
                g0 += sz

            # prep the first steady otiles while phase 1 streams (their
            # qw + broadcast DMAs queue behind the x tail on the ring)
            prep(PHI)
            prep(PHI + 1)

            # steady phase: otiles 4..n_otiles-1 sequential, mc-interleaved
            AHEAD = 2
            for ot in range(PHI, n_ot):
                nxt = ot + AHEAD
                if nxt < n_ot and nxt >= PHI + AHEAD:
                    prep(nxt)
                if ot == n_ot - 1:
                    # chain-major in 256-column quarter-chains, each in its
                    # own PSUM bank and evacuated immediately: the exposed
                    # tail after the very last matmul is a single 256-col
                    # ACT + store on the (empty by now) sync HWDGE ring
                    o0 = ot * 128
                    ob = min(128, oc - o0)
                    for q in range(2 * n_mch):
                        msl = slice(q * 256, q * 256 + 256)
                        ps_q = pspool.tile(
                            [128, 256], F32, name=f"ps_q{q}", tag="ps"
                        )
                        for gi in range(g):
                            for a, b, w3_t in w3s[ot]:
                                if a <= gi < a + b:
                                    j = gi - a
                                    break
                            nc.tensor.matmul(
                                ps_q[:],
                                w3_t[:, j * 128 : (j + 1) * 128],
                                xT_sb[:, gi, msl],
                                start=(gi == 0),
                                stop=(gi == g - 1),
                            )
                        o_t = opool.tile([128, MM_N], F16, name="o_t", tag="o")
                        nc.scalar.activation(
                            o_t[:ob, :256],
                            ps_q[:ob],
                            mybir.ActivationFunctionType.Identity,
                            bias=cb_sb[:ob, ot : ot + 1],
                            scale=1.0,
                        )
                        nc.sync.dma_start(outT[o0 : o0 + ob, msl], o_t[:ob, :256])
                else:
                    pss = make_pss(ot)
                    for gi in range(g):
                        for mc in range(n_mch):
                            mm_one(ot, pss, gi, mc)
                    evac(ot, pss)

    nc.compile()
    return nc


def _get_nc(m=M, k=IN, oc=OC, n_cores=N_CORES):
    key = (m, k, oc, n_cores)
    if key not in _CACHE:
        _CACHE[key] = _build(*key)
    return _CACHE[key]


def _make_in_maps(x, qweight, qzeros, scales, bias, n_cores=N_CORES):
    iw8 = _unpack_int4(qweight)  # [IN, OUT] uint8
    iz8 = _unpack_int4(qzeros)  # [G, OUT] uint8
    kk, mm = x.shape[1], x.shape[0]
    g = kk // GS
    # [p, group, m]: partition = k-within-group, contiguous per-partition
    x3 = np.ascontiguousarray(x.T.reshape(g, GS, mm).transpose(1, 0, 2))
    oc = qweight.shape[1] * PACK // n_cores
    n_ot = (oc + 127) // 128
    ocp = n_ot * 128

    in_maps = []
    for c in range(n_cores):
        sl = slice(c * oc, (c + 1) * oc)
        wp = np.pad(iw8[:, sl], [(0, 0), (0, ocp - oc)])  # [IN, ocp]
        # qwT[p, ot, (g, o)] = wp[g*GS + p, ot*128 + o]
        qwT = np.ascontiguousarray(
            wp.reshape(g, GS, n_ot, 128).transpose(1, 2, 0, 3).reshape(GS, n_ot, -1)
        )
        sp = np.pad(scales[:, sl].astype(np.float16), [(0, 0), (0, ocp - oc)])
        s2 = np.ascontiguousarray(
            sp.reshape(g, n_ot, 128).transpose(1, 0, 2).reshape(n_ot, -1)
        )
        zp = np.pad(iz8[:, sl], [(0, 0), (0, ocp - oc)])
        zq2 = np.ascontiguousarray(
            zp.reshape(g, n_ot, 128).transpose(1, 0, 2).reshape(n_ot, -1)
        )
        bp = np.pad(bias[sl].astype(np.float16), (0, ocp - oc))
        cb = np.ascontiguousarray(bp.reshape(n_ot, 128).T)
        in_maps.append({"x3": x3, "qwT": qwT, "s2": s2, "zq2": zq2, "cb": cb})
    return in_maps


LAST_EXEC_NS = None


def kernel(x, qweight, qzeros, scales, bias):
    global LAST_EXEC_NS
    from concourse.bass_utils import run_bass_kernel_spmd

    x = np.asarray(x)
    qweight = np.asarray(qweight)
    qzeros = np.asarray(qzeros)
    scales = np.asarray(scales)
    bias = np.asarray(bias)

    nc = _get_nc()
    in_maps = _make_in_maps(x, qweight, qzeros, scales, bias)

    kwargs = {}
    if os.environ.get("AWQ_PROFILE"):
        _enable_profiling()
        kwargs = dict(trace=True, tmpdir=os.environ.get("AWQ_TRACE_DIR") or None)
    res = run_bass_kernel_spmd(nc, in_maps, list(range(N_CORES)), **kwargs)
    LAST_EXEC_NS = res.exec_time_ns

    outT = np.concatenate([res.results[c]["outT"] for c in range(N_CORES)], axis=0)
    return np.ascontiguousarray(outT.T)


def _enable_profiling():
    """Register the NTFF profile hook missing from this image's antenv."""
    import types

    if "antenv.axon_hooks" not in sys.modules:
        import antenv

        mod = types.ModuleType("antenv.axon_hooks")
        mod._hook = None
        mod.set_axon_ntff_profile_hook = lambda h: setattr(mod, "_hook", h)
        mod.get_axon_ntff_profile_hook = lambda: mod._hook
        sys.modules["antenv.axon_hooks"] = mod
        antenv.axon_hooks = mod
        try:
            from trn_agent_boot.trn_boot import _ntff_profile_via_ctypes

            mod.set_axon_ntff_profile_hook(
                _ntff_profile_via_ctypes("/opt/axon/libaxon_pjrt.so")
            )
        except Exception:
            pass
    import concourse.bass_utils as _bu

    _bu.upload_artifacts = lambda tmpdir: "local://skipped"
